# revision 25
# baseline (speedup 1.0000x reference)
"""MiniMax-M2 MoE kernel for 8 Trainium2 NeuronCores — fp8 DoubleRow edition.

Single-launch expert-parallel design:
  Host (data movement / dispatch only): fp32 routing decides WHICH tokens go
    to WHICH expert (indices only); a planner cuts each expert's token list
    into at most two pieces and packs them into static expert slots per core;
    tokens are gathered per 256-token chunk and weights/activations are
    quantized to fp8(e4m3) hi+lo residual pairs sharing one scaled domain
    (hi = Q(v*s), lo = Q(v*s - hi)), so all residual-product terms accumulate
    raw in one PSUM group.
  Device (all output-value arithmetic, one SPMD launch):
    - matmuls run as fp8 DoubleRow (2 k-slices per instruction, 0.5 cyc/row)
      with 3 residual terms (hi*hi + lo*hi + hi*lo) per logical matmul —
      ~0.75x the bf16 cycle cost with bf16-level accuracy,
    - per chunk, router scores are recomputed on device from the same fp8
      pairs (3-term logits -> sigmoid -> top-4 threshold on bias-corrected
      scores -> renormalized combine weight of the slot's own expert),
    - stage 1 (SwiGLU) evacuates h in fp8 hi+lo pairs; stage 2 runs
      h-on-free / H-on-partitions so matmul cost tracks the real token count,
      scaled by combine weights broadcast via a PE transpose + rank-1 matmul,
    - stage 2 runs one chunk behind stage 1 (software pipeline).
  Host: scatter-add per-chunk outputs into [T, H].
"""

import math

import ml_dtypes
import numpy as np

import concourse.bass as bass
import concourse.tile as tile
from concourse import bacc, mybir
from concourse.bass_utils import run_bass_kernel_spmd

T, H, F, E, TOPK = 4096, 1024, 512, 16, 4
NCORES = 8
KC = H // 128    # contraction chunks (hidden dim)
FC = F // 128    # stage-2 contraction chunks
CH = 256         # tokens per chunk (DoubleRow moving-free limit)
F32 = mybir.dt.float32
BF16 = mybir.dt.bfloat16
FP8 = mybir.dt.float8e4
NF8 = ml_dtypes.float8_e4m3
DR = mybir.MatmulPerfMode.DoubleRow

# static pow2 scales (validated against the e4m3 max-normal 240 on the
# reference distribution; quantization is clipped on host anyway)
SX = 32.0     # hidden_states
SW = 1024.0   # w1/w3/w2
SG = 1024.0   # router gate
SH = 8.0      # h = silu(g)*u
INV1 = 1.0 / (SX * SW)    # psum(g) -> true g, for the Silu activation
C_H = INV1 * SH           # psum(u) * C_H * silu(g) = h*SH
INVR = 1.0 / (SX * SG)    # router logits psum -> true logits
INV_Y = 1.0 / (SH * SW)   # folded into the combine weights

_nc_cache: dict = {}
LAST_CAPS = (832, 492, 512, 354)


# Good general cap vectors found by offline search on the canonical routing
# distribution; each is validated against the ACTUAL counts at runtime (DP
# feasibility + placement construction) before use.
_CAPS_CANDIDATES = [(684, 604, 460, 350)]


def _dp_assign(caps, counts_desc):
    """Assign each expert (counts desc) a pair of slot types (i<=j) such
    that caps[i]+caps[j] >= count and each type is used at most 8 times.
    Returns the choice list or None."""
    pairs = [(i, j) for i in range(len(caps)) for j in range(i, len(caps))]
    capsum = {p: caps[p[0]] + caps[p[1]] for p in pairs}
    opts = []
    for c in counts_desc:
        o = [p for p in pairs if capsum[p] >= c]
        if not o:
            return None
        opts.append(o)
    n = len(counts_desc)
    seen = set()
    choice = [None] * n

    def dfs(k, rem):
        if k == n:
            return True
        key = (k, rem)
        if key in seen:
            return False
        for (i, j) in opts[k]:
            r2 = list(rem)
            r2[i] -= 1
            r2[j] -= 1
            if r2[i] >= 0 and r2[j] >= 0:
                choice[k] = (i, j)
                if dfs(k + 1, tuple(r2)):
                    return True
        seen.add(key)
        return False

    if not dfs(0, (NCORES,) * len(caps)):
        return None
    return choice


def _place_from_choice(caps, experts_desc, counts, choice):
    """Build placement[core][slot] = (expert, tok_start, len) from a
    type-pair assignment; both pieces of one expert land on distinct cores.
    Returns placement or None."""
    S = len(caps)
    pieces_per_type = [[] for _ in range(S)]
    for k, e in enumerate(experts_desc):
        i, j = choice[k]
        c = int(counts[e])
        pi = min(caps[i], c)
        pj = c - pi
        pieces_per_type[i].append((e, 0, pi))
        pieces_per_type[j].append((e, pi, pj))
    for rot in range(NCORES):
        placement = [[None] * S for _ in range(NCORES)]
        ok = True
        for t in range(S):
            free = list(range(NCORES))
            free = free[rot:] + free[:rot]
            for (e, st, ln) in sorted(pieces_per_type[t],
                                      key=lambda p: -p[2]):
                cand = [ci for ci in free
                        if e not in {p[0] for p in placement[ci] if p}]
                if not cand:
                    ok = False
                    break
                ci = cand[0]
                placement[ci][t] = (e, st, ln)
                free.remove(ci)
            if not ok:
                break
        if ok:
            return placement
    return None


def _plan_slots(counts: np.ndarray):
    """Choose per-core slot capacities and expert-piece placement.

    Experts are cut into at most two pieces assigned to a pair of slot
    types.  First the precomputed general cap vectors are tried (exact DP
    feasibility on the actual counts); otherwise a threshold-cut search
    (heavy/light primaries + ranked remainders) provides the fallback.

    Returns (caps, placement) where placement[core] is a list of
    (expert, tok_start, length) per slot (length may be 0).
    """
    E_ = len(counts)
    order = np.argsort(-counts, kind="stable")
    heavy = [int(e) for e in order[:NCORES]]
    light = [int(e) for e in order[NCORES:]]
    c0 = int(counts[heavy[0]])
    c8 = int(counts[light[0]])

    def plan_cost(caps):
        ct = sum(caps)
        nch = sum(math.ceil(cp / CH) for cp in caps if cp)
        return 75 * ct + 900 * nch

    def build(A, C):
        pieces_b = []  # (expert, start, len) remainders
        for e in heavy:
            if counts[e] > A:
                pieces_b.append((e, A, int(counts[e]) - A))
        for e in light:
            if counts[e] > C:
                pieces_b.append((e, C, int(counts[e]) - C))
        if len(pieces_b) > 2 * NCORES:
            return None
        pieces_b.sort(key=lambda p: -p[2])
        bs = pieces_b[:NCORES]
        ds = pieces_b[NCORES:]
        a = min(c0, A)
        b = bs[0][2] if bs else 0
        c = min(c8, C)
        d = ds[0][2] if ds else 0
        caps = (a, b, c, d)
        placement = [[None] * 4 for _ in range(NCORES)]
        for i in range(NCORES):
            placement[i][0] = (heavy[i], 0, min(int(counts[heavy[i]]), A))
            placement[i][2] = (light[i], 0, min(int(counts[light[i]]), C))
        for sl, plist in ((1, bs), (3, ds)):
            free = set(range(NCORES))
            for e, st, ln in plist:
                cand = [i for i in free
                        if e != placement[i][0][0] and e != placement[i][2][0]
                        and (placement[i][1] is None or
                             placement[i][1][0] != e)]
                if not cand:
                    return None
                i = cand[0]
                free.discard(i)
                placement[i][sl] = (e, st, ln)
        return caps, placement

    best = None
    lo_a = (c0 + 1) // 2
    lo_c = (c8 + 1) // 2
    cands = [(c0, c8)]
    for A in range(lo_a, c0 + 1, 2):
        for C in range(lo_c, c8 + 1, 2):
            cands.append((A, C))
    for A, C in cands:
        got = build(A, C)
        if got is None:
            continue
        caps, placement = got
        cost = plan_cost(caps)
        if best is None or cost < best[0]:
            best = (cost, caps, placement)
    experts_desc = [int(e) for e in order]
    counts_desc = [int(counts[e]) for e in experts_desc]
    for caps_c in _CAPS_CANDIDATES:
        if plan_cost(caps_c) >= best[0]:
            continue
        choice = _dp_assign(caps_c, counts_desc)
        if choice is None:
            continue
        pl = _place_from_choice(caps_c, experts_desc, counts, choice)
        if pl is None:
            continue
        best = (plan_cost(caps_c), caps_c, pl)
    _, caps, placement = best
    # drop zero-cap slots; fill empty kept slots with a zero-length piece of
    # some expert not already used by that core (perm needs distinct experts)
    keep = [si for si in range(4) if caps[si] > 0]
    caps_k = tuple(caps[si] for si in keep)
    placement_k = []
    for pl in placement:
        row = []
        used = {p[0] for p in pl if p is not None}
        for si in keep:
            p = pl[si]
            if p is None:
                e_fill = next(e for e in range(E_) if e not in used)
                used.add(e_fill)
                p = (e_fill, 0, 0)
            row.append(p)
        placement_k.append(row)
    return caps_k, placement_k


def _chunk_sizes(cap: int) -> list[int]:
    """Split cap into <=256-sized chunks; remainder last."""
    n_full, rem = divmod(cap, CH)
    sizes = [CH] * n_full
    if rem:
        sizes = sizes + [rem]
    return sizes


def _chunk_table(caps):
    """Global chunk list: (slot, t0_in_slot, tl)."""
    out = []
    for s, cap in enumerate(caps):
        t0 = 0
        for tl in _chunk_sizes(cap):
            out.append((s, t0, tl))
            t0 += tl
    return out


def _build_moe(caps: tuple[int, ...]):
    """One-launch fp8 DoubleRow MoE FFN + on-device combine weights.

    Inputs per core (S = len(caps) expert slots, NCH = chunk count):
      w13h/w13l [S, H, 2F] fp8   hi/lo of hstack(w1[e].T, w3[e].T) * SW
      w2h/w2l   [S, F, H]  fp8   hi/lo of w2[e].T * SW
      xph/xpl   [128, NCH, KC, 256] fp8  gathered tokens * SX, packed per
                                   chunk in the SBUF image layout
      gth/gtl   [H, E] fp8       gate_w.T * SG, columns permuted so column s
                                 is slot s's expert
      biasp     [128, E] f32     e_score_correction_bias, same permutation
      identb    [128, 128] bf16  identity (PE transpose)
    Output:
      ygp [128, NCH, FC*2, 256] bf16  combine-weighted expert outputs,
                                 H-on-partitions: ygp[p, c, hk, t] =
                                 y[token t of chunk c, hk*128 + p]
    """
    S = len(caps)
    chunks = _chunk_table(caps)
    NCH = len(chunks)
    ntiles_total = sum(math.ceil(tl / 128) for _, _, tl in chunks)
    HK = H // 128

    nc = bacc.Bacc("TRN2", target_bir_lowering=False, debug=False,
                   num_devices=NCORES)
    w13h = nc.dram_tensor("w13h", [S, 128, KC * 2 * F], FP8,
                          kind="ExternalInput").ap()
    w13l = nc.dram_tensor("w13l", [S, 128, KC * 2 * F], FP8,
                          kind="ExternalInput").ap()
    w2h = nc.dram_tensor("w2h", [S, 128, FC * H], FP8,
                         kind="ExternalInput").ap()
    w2l = nc.dram_tensor("w2l", [S, 128, FC * H], FP8,
                         kind="ExternalInput").ap()
    xph = nc.dram_tensor("xph", [128, NCH, KC * CH], FP8,
                         kind="ExternalInput").ap()
    xpl = nc.dram_tensor("xpl", [128, NCH, KC * CH], FP8,
                         kind="ExternalInput").ap()
    gth = nc.dram_tensor("gth", [128, KC, E], FP8,
                         kind="ExternalInput").ap()
    gtl = nc.dram_tensor("gtl", [128, KC, E], FP8,
                         kind="ExternalInput").ap()
    biasp = nc.dram_tensor("biasp", [128, E], F32, kind="ExternalInput").ap()
    identb = nc.dram_tensor("identb", [128, 128], BF16,
                            kind="ExternalInput").ap()
    ygp = nc.dram_tensor("ygp", [128, NCH, HK * CH], BF16,
                         kind="ExternalOutput").ap()

    SIG = mybir.ActivationFunctionType.Sigmoid
    SILU = mybir.ActivationFunctionType.Silu
    COPY = mybir.ActivationFunctionType.Copy
    MUL = mybir.AluOpType.mult
    ADD = mybir.AluOpType.add

    with tile.TileContext(nc) as tc:
        with (
            tc.tile_pool(name="const_p", bufs=1) as const_p,
            tc.tile_pool(name="w13_p", bufs=2) as w13_p,
            tc.tile_pool(name="w2_p", bufs=2) as w2_p,
            tc.tile_pool(name="xg_p", bufs=3) as xg_p,
            tc.tile_pool(name="ht_p", bufs=2) as ht_p,
            tc.tile_pool(name="work_p", bufs=2) as work_p,
            tc.tile_pool(name="y_p", bufs=3) as y_p,
            tc.tile_pool(name="ps1", bufs=4, space="PSUM") as ps1,
            tc.tile_pool(name="psy", bufs=2, space="PSUM") as psy,
            tc.tile_pool(name="psmisc", bufs=2, space="PSUM") as psmisc,
        ):
            gt_h = const_p.tile([128, KC, E], FP8)
            gt_l = const_p.tile([128, KC, E], FP8)
            bias_sb = const_p.tile([128, E], F32)
            ident_sb = const_p.tile([128, 128], BF16)
            ones_sb = const_p.tile([1, 128], BF16)
            w_sb = const_p.tile([128, ntiles_total], BF16)
            nc.gpsimd.memset(ones_sb[:], 1.0)

            # ---------------- device-side emission helpers ----------------
            def routing(xh, xl, tl, nt, s, cg, jg):
                """Combine weight (bf16, *INV_Y) of slot s's expert for one
                chunk; token-on-partition orientation."""
                ps_r = psmisc.tile([128, 2, E], F32, tag="psmisc",
                                   name=f"ps_r_{cg}")
                nc.vector.memset(ps_r[:, :nt, :], 0.0)
                for j in range(nt):
                    tt0 = j * 128
                    ttl = min(128, tl - tt0)
                    i = 0
                    for (X, G) in ((xh, gt_h), (xl, gt_h), (xh, gt_l)):
                        for kp in range(KC // 2):
                            nc.tensor.matmul(
                                ps_r[:ttl, j, :],
                                lhsT=X[:, 2 * kp:2 * kp + 2, tt0:tt0 + ttl],
                                rhs=G[:, 2 * kp:2 * kp + 2, :],
                                start=(i == 0), stop=(i == 3 * KC // 2 - 1),
                                perf_mode=DR)
                            i += 1
                # sigmoid via tanh: scores = 0.5 + 0.5*tanh(l/2); Tanh shares
                # the activation table with Silu (no LoadActFuncSet thrash)
                th = work_p.tile([128, 2, E], F32, tag="th",
                                 name=f"th_{cg}")
                nc.scalar.activation(th[:, :nt, :], ps_r[:, :nt, :],
                                     mybir.ActivationFunctionType.Tanh,
                                     scale=INVR * 0.5)
                sc = work_p.tile([128, 2, E], F32, tag="sc",
                                 name=f"sc_{cg}")
                nc.vector.tensor_scalar(
                    sc[:, :nt, :], th[:, :nt, :], 0.5, 0.5,
                    op0=MUL, op1=ADD)
                biased = work_p.tile([128, 2, E], F32, tag="biased",
                                     name=f"biased_{cg}")
                nc.vector.tensor_tensor(
                    biased[:, :nt, :], sc[:, :nt, :],
                    bias_sb[:, None, :].to_broadcast([128, nt, E]),
                    ADD)
                m8 = work_p.tile([128, 2, 8], F32, tag="m8",
                                 name=f"m8_{cg}")
                sel = work_p.tile([128, 2, E], F32, tag="sel",
                                  name=f"sel_{cg}")
                for j in range(nt):
                    nc.vector.max(m8[:, j, :], biased[:, j, :])
                for j in range(nt):
                    nc.vector.tensor_scalar(
                        sel[:, j, :], biased[:, j, :],
                        m8[:, j, TOPK - 1:TOPK], None,
                        op0=mybir.AluOpType.is_ge)
                picked = work_p.tile([128, 2, E], F32, tag="picked",
                                     name=f"picked_{cg}")
                nc.vector.tensor_mul(
                    picked[:, :nt, :], sel[:, :nt, :], sc[:, :nt, :])
                denom = work_p.tile([128, 2], F32, tag="denom",
                                    name=f"denom_{cg}")
                nc.vector.reduce_sum(
                    denom[:, :nt], picked[:, :nt, :],
                    axis=mybir.AxisListType.X)
                recip = work_p.tile([128, 2], F32, tag="recip",
                                    name=f"recip_{cg}")
                nc.vector.reciprocal(recip[:, :nt], denom[:, :nt])
                nc.vector.scalar_tensor_tensor(
                    w_sb[:, jg:jg + nt], sc[:, :nt, s], INV_Y,
                    recip[:, :nt], MUL, MUL)

            def evac_stage1(ps_g, ps_u, hth, htl, fi, tl, cg):
                """h*SH in fp8 hi+lo: hi = Q(t), lo = Q(t - hi)."""
                sg = work_p.tile([128, CH], F32, tag="sg",
                                 name=f"sg_{cg}_{fi}")
                nc.scalar.activation(sg[:, :tl], ps_g[:, :tl], SILU,
                                     scale=INV1)
                tt = work_p.tile([128, CH], F32, tag="tt",
                                 name=f"tt_{cg}_{fi}")
                nc.vector.scalar_tensor_tensor(
                    tt[:, :tl], ps_u[:, :tl], C_H, sg[:, :tl], MUL, MUL)
                # Pool does the fp8 cast + lo residual (SBUF-only operands)
                nc.gpsimd.tensor_copy(hth[:, fi, :tl], tt[:, :tl])
                nc.gpsimd.tensor_tensor(
                    htl[:, fi, :tl], tt[:, :tl], hth[:, fi, :tl],
                    mybir.AluOpType.subtract)

            def stage1_fi(ps_g, ps_u, xh, xl, w13h_sb, w13l_sb, fi, tl):
                """ps_g/ps_u accumulate 3 residual terms x 4 k-pairs."""
                i = 0
                for (X, W) in ((xh, w13h_sb), (xl, w13h_sb), (xh, w13l_sb)):
                    for kp in range(KC // 2):
                        nc.tensor.matmul(
                            ps_g[:, :tl],
                            lhsT=W[:, 2 * kp:2 * kp + 2,
                                   fi * 128:(fi + 1) * 128],
                            rhs=X[:, 2 * kp:2 * kp + 2, :tl],
                            start=(i == 0), stop=(i == 3 * KC // 2 - 1),
                            perf_mode=DR)
                        i += 1
                i = 0
                for (X, W) in ((xh, w13h_sb), (xl, w13h_sb), (xh, w13l_sb)):
                    for kp in range(KC // 2):
                        nc.tensor.matmul(
                            ps_u[:, :tl],
                            lhsT=W[:, 2 * kp:2 * kp + 2,
                                   F + fi * 128:F + (fi + 1) * 128],
                            rhs=X[:, 2 * kp:2 * kp + 2, :tl],
                            start=(i == 0), stop=(i == 3 * KC // 2 - 1),
                            perf_mode=DR)
                        i += 1

            def make_wb(p):
                """Combine-weight broadcast along partitions (bf16):
                transpose w column -> rank-1 ones matmul."""
                tl, nt, cg, jg = p["tl"], p["nt"], p["cg"], p["jg"]
                wb_ps = psmisc.tile([128, CH], F32, tag="psmisc",
                                    name=f"wb_ps_{cg}")
                for j in range(nt):
                    wrow_ps = psmisc.tile([1, 128], BF16, tag="psmisc",
                                          name=f"wrow_ps_{cg}_{j}")
                    nc.tensor.transpose(
                        wrow_ps[0:1, :], w_sb[:, jg + j:jg + j + 1],
                        ident_sb[:])
                    wrow_sb = work_p.tile([1, 128], BF16, tag="wrow_sb",
                                          name=f"wrow_sb_{cg}_{j}")
                    nc.vector.tensor_copy(wrow_sb[0:1, :], wrow_ps[0:1, :])
                    nc.tensor.matmul(
                        wb_ps[:, j * 128:j * 128 + 128],
                        lhsT=ones_sb[0:1, :], rhs=wrow_sb[0:1, :],
                        start=True, stop=True)
                wb_sb = work_p.tile([128, CH], BF16, tag="wb_sb",
                                    name=f"wb_sb_{cg}")
                nc.scalar.activation(wb_sb[:, :tl], wb_ps[:, :tl], COPY)
                p["wb_sb"] = wb_sb

            def emit_stage2(p):
                """Stage 2 for one chunk (one behind): H-on-partitions,
                y[p + 128*hk, t] = wb[t] * sum_f h[f,t] w2T[f, p+128*hk]."""
                tl, nt, cg, jg = p["tl"], p["nt"], p["cg"], p["jg"]
                hth, htl = p["hth"], p["htl"]
                w2h_sb, w2l_sb = p["w2h"], p["w2l"]
                if "wb_sb" not in p:
                    make_wb(p)
                wb_sb = p["wb_sb"]
                y_sb = y_p.tile([128, HK, CH], BF16, tag="y",
                                name=f"y_sb_{cg}")
                if tl < CH:
                    # keep the DMA'd tail defined (full-chunk writes)
                    nc.gpsimd.memset(y_sb[:, :, tl:], 0.0)
                for hk in range(HK):
                    ps_y = psy.tile([128, CH], F32, tag="psy",
                                    name=f"ps_y_{cg}_{hk}")
                    i = 0
                    for (A, B) in ((w2h_sb, hth), (w2h_sb, htl),
                                   (w2l_sb, hth)):
                        for kfp in range(FC // 2):
                            nc.tensor.matmul(
                                ps_y[:, :tl],
                                lhsT=A[:, 2 * kfp:2 * kfp + 2,
                                       hk * 128:(hk + 1) * 128],
                                rhs=B[:, 2 * kfp:2 * kfp + 2, :tl],
                                start=(i == 0), stop=(i == 3 * FC // 2 - 1),
                                perf_mode=DR)
                            i += 1
                    if ((p.get("final", False) or p.get("penult", False))
                            and hk % 2 == 1):
                        yr = work_p.tile([128, CH], BF16, tag="yr",
                                         name=f"yr_{cg}_{hk}")
                        nc.scalar.activation(yr[:, :tl], ps_y[:, :tl], COPY)
                        nc.gpsimd.tensor_tensor(
                            y_sb[:, hk, :tl], yr[:, :tl], wb_sb[:, :tl],
                            MUL)
                    else:
                        nc.vector.tensor_tensor(
                            y_sb[:, hk, :tl], ps_y[:, :tl], wb_sb[:, :tl],
                            MUL)
                if p.get("final", False) or p.get("penult", False):
                    # drain fast: per-hk DMAs fire as soon as rows are ready,
                    # alternating issue queues
                    for hk in range(HK):
                        eng = nc.sync if hk % 2 == 0 else nc.scalar
                        eng.dma_start(ygp[:, cg, hk * CH:(hk + 1) * CH],
                                      y_sb[:, hk, :])
                    return None
                return (cg, y_sb)

            # ---------------- weight DMA piece scheduling ----------------
            def w13_tiles(s):
                th = w13_p.tile([128, KC, 2 * F], FP8, tag="w13h",
                                name=f"w13h_sb_{s}")
                tl_ = w13_p.tile([128, KC, 2 * F], FP8, tag="w13l",
                                 name=f"w13l_sb_{s}")
                return th, tl_

            def w2_tiles(s):
                th = w2_p.tile([128, FC, H], FP8, tag="w2h",
                               name=f"w2h_sb_{s}")
                tl_ = w2_p.tile([128, FC, H], FP8, tag="w2l",
                                name=f"w2l_sb_{s}")
                return th, tl_

            def weight_thunks(s, th13, tl13, th2, tl2, pieces=False):
                """DMA thunk list for slot s's weights, in issue order.
                pieces=True splits w13 per k-pair (ramp streaming)."""
                thunks = []
                PW = 2 * 2 * F
                if pieces:
                    for kp in range(KC // 2):
                        thunks.append(
                            lambda kp=kp: nc.gpsimd.dma_start(
                                th13[:, 2 * kp:2 * kp + 2, :].rearrange(
                                    "p k f -> p (k f)"),
                                w13h[s, :, kp * PW:(kp + 1) * PW]))
                    for kp in range(KC // 2):
                        thunks.append(
                            lambda kp=kp: nc.sync.dma_start(
                                tl13[:, 2 * kp:2 * kp + 2, :].rearrange(
                                    "p k f -> p (k f)"),
                                w13l[s, :, kp * PW:(kp + 1) * PW]))
                else:
                    thunks.append(lambda: nc.gpsimd.dma_start(
                        th13[:].rearrange("p k f -> p (k f)"), w13h[s]))
                    thunks.append(lambda: nc.scalar.dma_start(
                        th2[:].rearrange("p k h -> p (k h)"), w2h[s]))
                    thunks.append(lambda: nc.sync.dma_start(
                        tl13[:].rearrange("p k f -> p (k f)"), w13l[s]))
                    thunks.append(lambda: nc.scalar.dma_start(
                        tl2[:].rearrange("p k h -> p (k h)"), w2l[s]))
                    return thunks
                thunks.append(lambda: nc.scalar.dma_start(
                    th2[:].rearrange("p k h -> p (k h)"), w2h[s]))
                thunks.append(lambda: nc.scalar.dma_start(
                    tl2[:].rearrange("p k h -> p (k h)"), w2l[s]))
                return thunks

            # ---------------- main emission ----------------
            slot_w13 = {}
            slot_w2 = {}
            pending = None
            pending_y = None
            prefetch: list = []
            next_x = None
            jglob = 0
            for cg, (s, t0s, tl) in enumerate(chunks):
                nt = math.ceil(tl / 128)
                last_chunks_of_slot = sum(1 for c2 in chunks[cg:]
                                          if c2[0] == s)
                ramp = (cg == 0)

                if s == 0 and cg == 0:
                    slot_w13[0] = w13_tiles(0)
                    slot_w2[0] = w2_tiles(0)
                # s > 0: tiles were created when prefetch was scheduled
                w13h_sb, w13l_sb = slot_w13[s]
                w2h_sb, w2l_sb = slot_w2[s]

                if pending_y is not None:
                    ycg, y_prev = pending_y
                    nc.gpsimd.dma_start(
                        ygp[:, ycg, :],
                        y_prev[:].rearrange("p h c -> p (h c)"))
                    pending_y = None
                if next_x is not None:
                    xh, xl = next_x
                else:
                    xh = xg_p.tile([128, KC, CH], FP8, tag="xgh",
                                   name=f"xh_{cg}")
                    xl = xg_p.tile([128, KC, CH], FP8, tag="xgl",
                                   name=f"xl_{cg}")

                if ramp:
                    # ramp-critical stream: interleave x / w13h k-pair pieces
                    # so the hi-term matmuls start as early as possible
                    PW = 2 * 2 * F
                    PX = 2 * CH
                    for kp in range(KC // 2):
                        nc.sync.dma_start(
                            xh[:, 2 * kp:2 * kp + 2, :].rearrange(
                                "p k c -> p (k c)"),
                            xph[:, 0, kp * PX:(kp + 1) * PX])
                        nc.gpsimd.dma_start(
                            w13h_sb[:, 2 * kp:2 * kp + 2, :].rearrange(
                                "p k f -> p (k f)"),
                            w13h[0, :, kp * PW:(kp + 1) * PW])
                    for kp in range(KC // 2):
                        nc.sync.dma_start(
                            xl[:, 2 * kp:2 * kp + 2, :].rearrange(
                                "p k c -> p (k c)"),
                            xpl[:, 0, kp * PX:(kp + 1) * PX])
                        nc.scalar.dma_start(
                            w13l_sb[:, 2 * kp:2 * kp + 2, :].rearrange(
                                "p k f -> p (k f)"),
                            w13l[0, :, kp * PW:(kp + 1) * PW])
                    nc.sync.dma_start(gt_h[:], gth)
                    nc.sync.dma_start(gt_l[:], gtl)
                    nc.sync.dma_start(bias_sb[:], biasp)
                    nc.sync.dma_start(ident_sb[:], identb)
                    # stage 1 in two fi-phases (PSUM: 4 full banks each),
                    # k-outer inside so matmuls chase the DMA stream
                    for fis in ((0, 1), (2, 3)):
                        psg = {fi: ps1.tile([128, 512], F32, tag="ps1",
                                            name=f"ps_g_r{fi}")
                               for fi in fis}
                        psu = {fi: ps1.tile([128, 512], F32, tag="ps1",
                                            name=f"ps_u_r{fi}")
                               for fi in fis}
                        for ti, (X, W) in enumerate(
                                ((xh, w13h_sb), (xl, w13h_sb),
                                 (xh, w13l_sb))):
                            for kp in range(KC // 2):
                                for fi in fis:
                                    for half, ps in ((0, psg[fi]),
                                                     (1, psu[fi])):
                                        nc.tensor.matmul(
                                            ps[:, :tl],
                                            lhsT=W[:, 2 * kp:2 * kp + 2,
                                                   half * F + fi * 128:
                                                   half * F + (fi + 1) * 128],
                                            rhs=X[:, 2 * kp:2 * kp + 2, :tl],
                                            start=(ti == 0 and kp == 0),
                                            stop=(ti == 2 and
                                                  kp == KC // 2 - 1),
                                            perf_mode=DR)
                        if fis == (0, 1):
                            hth = ht_p.tile([128, FC, CH], FP8, tag="hth",
                                            name=f"hth_{cg}")
                            htl = ht_p.tile([128, FC, CH], FP8, tag="htl",
                                            name=f"htl_{cg}")
                        for fi in fis:
                            evac_stage1(psg[fi], psu[fi], hth, htl,
                                        fi, tl, cg)
                        if fis == (0, 1):
                            # slot-0 w2 behind the first silu/cast chain
                            nc.scalar.dma_start(
                                w2h_sb[:].rearrange("p k h -> p (k h)"),
                                w2h[0])
                            nc.scalar.dma_start(
                                w2l_sb[:].rearrange("p k h -> p (k h)"),
                                w2l[0])
                    routing(xh, xl, tl, nt, s, cg, jglob)
                else:
                    routing(xh, xl, tl, nt, s, cg, jglob)
                    hth = ht_p.tile([128, FC, CH], FP8, tag="hth",
                                    name=f"hth_{cg}")
                    htl = ht_p.tile([128, FC, CH], FP8, tag="htl",
                                    name=f"htl_{cg}")
                    for fi in range(2):
                        ps_g = ps1.tile([128, 512], F32, tag="ps1",
                                        name=f"ps_g_{cg}_{fi}")
                        ps_u = ps1.tile([128, 512], F32, tag="ps1",
                                        name=f"ps_u_{cg}_{fi}")
                        stage1_fi(ps_g, ps_u, xh, xl, w13h_sb, w13l_sb,
                                  fi, tl)
                        evac_stage1(ps_g, ps_u, hth, htl, fi, tl, cg)
                    if pending is not None:
                        pending_y = emit_stage2(pending)
                    for fi in range(2, 4):
                        ps_g = ps1.tile([128, 512], F32, tag="ps1",
                                        name=f"ps_g_{cg}_{fi}")
                        ps_u = ps1.tile([128, 512], F32, tag="ps1",
                                        name=f"ps_u_{cg}_{fi}")
                        stage1_fi(ps_g, ps_u, xh, xl, w13h_sb, w13l_sb,
                                  fi, tl)
                        evac_stage1(ps_g, ps_u, hth, htl, fi, tl, cg)

                # prefetch: next chunk's tokens (SP queue)
                if cg + 1 < NCH:
                    xh_n = xg_p.tile([128, KC, CH], FP8, tag="xgh",
                                     name=f"xh_{cg + 1}")
                    xl_n = xg_p.tile([128, KC, CH], FP8, tag="xgl",
                                     name=f"xl_{cg + 1}")
                    nc.sync.dma_start(
                        xh_n[:].rearrange("p k c -> p (k c)"),
                        xph[:, cg + 1, :])
                    nc.sync.dma_start(
                        xl_n[:].rearrange("p k c -> p (k c)"),
                        xpl[:, cg + 1, :])
                    next_x = (xh_n, xl_n)
                else:
                    next_x = None

                # prefetch: next slot's weights, spread over this slot's
                # remaining chunks (ACT queue)
                nxt = s + 1
                if nxt < S and nxt not in slot_w13:
                    slot_w13[nxt] = w13_tiles(nxt)
                    slot_w2[nxt] = w2_tiles(nxt)
                    prefetch = weight_thunks(nxt, *slot_w13[nxt],
                                             *slot_w2[nxt])
                if prefetch:
                    npop = math.ceil(len(prefetch) / last_chunks_of_slot) \
                        if last_chunks_of_slot else len(prefetch)
                    for t in prefetch[:npop]:
                        t()
                    prefetch = prefetch[npop:]

                pending = {"tl": tl, "nt": nt, "cg": cg, "jg": jglob,
                           "hth": hth, "htl": htl, "final": cg == NCH - 1,
                           "penult": cg == NCH - 2,
                           "w2h": w2h_sb, "w2l": w2l_sb}
                jglob += nt
                if cg == NCH - 1:
                    # final chunk: wb build as soon as its routing chain is
                    # done (during its own stage 1) to shorten the drain
                    make_wb(pending)

            if pending_y is not None:
                ycg, y_prev = pending_y
                nc.scalar.dma_start(
                    ygp[:, ycg, :],
                    y_prev[:].rearrange("p h c -> p (h c)"))
            emit_stage2(pending)

    nc.compile()
    return nc


def _moe_nc(caps):
    key = ("moe8", caps)
    if key not in _nc_cache:
        _nc_cache[key] = _build_moe(caps)
    return _nc_cache[key]


def _split_fp8(v: np.ndarray, s: float):
    """hi = Q(clip(v*s)), lo = Q(v*s - hi); shared scaled domain."""
    vs = v * s
    hi = np.clip(vs, -240.0, 240.0).astype(NF8)
    lo = (vs - hi.astype(np.float32)).astype(NF8)
    return hi, lo


def kernel(hidden_states, gate_w, bias, w1, w3, w2):
    x = np.ascontiguousarray(np.asarray(hidden_states, dtype=np.float32))
    gate_w = np.asarray(gate_w, dtype=np.float32)
    bias = np.asarray(bias, dtype=np.float32)
    w1 = np.asarray(w1, dtype=np.float32)
    w3 = np.asarray(w3, dtype=np.float32)
    w2 = np.asarray(w2, dtype=np.float32)

    # ---- Host dispatch: fp32 routing decides token->expert placement ----
    logits = x @ gate_w.T                                # [T, E]
    scores = 1.0 / (1.0 + np.exp(-logits))
    biased = scores + bias[None, :]
    topi = np.argpartition(-biased, TOPK - 1, axis=1)[:, :TOPK]  # [T, K]
    sel = np.zeros((T, E), dtype=bool)
    sel[np.arange(T)[:, None], topi] = True
    idx_per_e = [np.nonzero(sel[:, e])[0] for e in range(E)]
    counts = np.array([len(ix) for ix in idx_per_e])
    caps, placement = _plan_slots(counts)
    S = len(caps)
    offs = [sum(caps[:si]) for si in range(S)]
    global LAST_CAPS
    LAST_CAPS = caps
    CT = sum(caps)
    chunks = _chunk_table(caps)
    NCH = len(chunks)

    xT = np.ascontiguousarray(x.T)                       # [H, T]
    gT = np.ascontiguousarray(gate_w.T)                  # [H, E]

    in_maps = []
    for c in range(NCORES):
        slot_experts = [p[0] for p in placement[c]]
        idx_pad = np.zeros(CT, dtype=np.int64)
        for si, (e, st, ln) in enumerate(placement[c]):
            if ln:
                idx_pad[offs[si]:offs[si] + ln] = idx_per_e[e][st:st + ln]
        xg = xT[:, idx_pad]                              # [H, CT] f32
        xg_hi, xg_lo = _split_fp8(xg, SX)
        xph = np.zeros((128, NCH, KC, CH), dtype=NF8)
        xpl = np.zeros((128, NCH, KC, CH), dtype=NF8)
        for cg, (si, t0, tl) in enumerate(chunks):
            colr = slice(offs[si] + t0, offs[si] + t0 + tl)
            xph[:, cg, :, :tl] = \
                xg_hi[:, colr].reshape(KC, 128, tl).transpose(1, 0, 2)
            xpl[:, cg, :, :tl] = \
                xg_lo[:, colr].reshape(KC, 128, tl).transpose(1, 0, 2)

        w13 = np.stack([
            np.concatenate([w1[e].T, w3[e].T], axis=1)
            for e in slot_experts])                      # [S, H, 2F]
        # partition-major SBUF image: [S, 128, KC, 2F]
        w13 = np.ascontiguousarray(
            w13.reshape(S, KC, 128, 2 * F).transpose(0, 2, 1, 3))
        w13h_a, w13l_a = _split_fp8(w13, SW)
        w2t = np.stack([w2[e].T for e in slot_experts])  # [S, F, H]
        w2t = np.ascontiguousarray(
            w2t.reshape(S, FC, 128, H).transpose(0, 2, 1, 3))
        w2h_a, w2l_a = _split_fp8(w2t, SW)

        perm = slot_experts + [e for e in range(E) if e not in slot_experts]
        gtp = np.ascontiguousarray(
            gT[:, perm].reshape(KC, 128, E).transpose(1, 0, 2))
        gth_a, gtl_a = _split_fp8(gtp, SG)
        biasp = np.ascontiguousarray(
            np.broadcast_to(np.asarray(bias)[perm][None, :],
                            (128, E))).astype(np.float32)
        in_maps.append({
            "w13h": w13h_a.reshape(S, 128, KC * 2 * F),
            "w13l": w13l_a.reshape(S, 128, KC * 2 * F),
            "w2h": w2h_a.reshape(S, 128, FC * H),
            "w2l": w2l_a.reshape(S, 128, FC * H),
            "xph": xph.reshape(128, NCH, KC * CH),
            "xpl": xpl.reshape(128, NCH, KC * CH),
            "gth": gth_a, "gtl": gtl_a,
            "biasp": biasp, "identb": np.eye(128, dtype=ml_dtypes.bfloat16),
        })

    # ---- Single SPMD launch: routing weights + expert FFN ----
    ncB = _moe_nc(caps)
    res = run_bass_kernel_spmd(ncB, in_maps, core_ids=list(range(NCORES)))

    # ---- Host combine: scatter-add ----
    out = np.zeros((T, H), dtype=np.float32)
    for c in range(NCORES):
        ygp_c = res.results[c]["ygp"].reshape(128, NCH, H // 128, CH)
        for cg, (si, t0, tl) in enumerate(chunks):
            e, st, ln = placement[c][si]
            cnt = min(max(ln - t0, 0), tl)
            if cnt <= 0:
                continue
            ix = idx_per_e[e][st + t0:st + t0 + cnt]
            blk = ygp_c[:, cg, :, :cnt].astype(np.float32)   # [128, 8, cnt]
            out[ix] += blk.transpose(2, 1, 0).reshape(cnt, H)
    return out


# revision 26
# speedup vs baseline: 1.0338x; 1.0338x over previous
"""MiniMax-M2 MoE kernel for 8 Trainium2 NeuronCores — fp8 DoubleRow edition.

Single-launch expert-parallel design:
  Host (data movement / dispatch only): fp32 routing decides WHICH tokens go
    to WHICH expert (indices only); a planner cuts each expert's token list
    into at most two pieces and packs them into static expert slots per core;
    tokens are gathered per 256-token chunk and weights/activations are
    quantized to fp8(e4m3) hi+lo residual pairs sharing one scaled domain
    (hi = Q(v*s), lo = Q(v*s - hi)), so all residual-product terms accumulate
    raw in one PSUM group.
  Device (all output-value arithmetic, one SPMD launch):
    - matmuls run as fp8 DoubleRow (2 k-slices per instruction, 0.5 cyc/row)
      with 3 residual terms (hi*hi + lo*hi + hi*lo) per logical matmul —
      ~0.75x the bf16 cycle cost with bf16-level accuracy,
    - per chunk, router scores are recomputed on device from the same fp8
      pairs (3-term logits -> sigmoid -> top-4 threshold on bias-corrected
      scores -> renormalized combine weight of the slot's own expert),
    - stage 1 (SwiGLU) evacuates h in fp8 hi+lo pairs; stage 2 runs
      h-on-free / H-on-partitions so matmul cost tracks the real token count,
      scaled by combine weights broadcast via a PE transpose + rank-1 matmul,
    - stage 2 runs one chunk behind stage 1 (software pipeline).
  Host: scatter-add per-chunk outputs into [T, H].
"""

import math

import ml_dtypes
import numpy as np

import concourse.bass as bass
import concourse.tile as tile
from concourse import bacc, mybir
from concourse.bass_utils import run_bass_kernel_spmd

T, H, F, E, TOPK = 4096, 1024, 512, 16, 4
NCORES = 8
KC = H // 128    # contraction chunks (hidden dim)
FC = F // 128    # stage-2 contraction chunks
CH = 256         # tokens per chunk (DoubleRow moving-free limit)
F32 = mybir.dt.float32
BF16 = mybir.dt.bfloat16
FP8 = mybir.dt.float8e4
NF8 = ml_dtypes.float8_e4m3
DR = mybir.MatmulPerfMode.DoubleRow

# static pow2 scales (validated against the e4m3 max-normal 240 on the
# reference distribution; quantization is clipped on host anyway)
SX = 32.0     # hidden_states
SW = 1024.0   # w1/w3/w2
SG = 1024.0   # router gate
SH = 8.0      # h = silu(g)*u
INV1 = 1.0 / (SX * SW)    # psum(g) -> true g, for the Silu activation
C_H = INV1 * SH           # psum(u) * C_H * silu(g) = h*SH
INVR = 1.0 / (SX * SG)    # router logits psum -> true logits
INV_Y = 1.0 / (SH * SW)   # folded into the combine weights

_nc_cache: dict = {}
LAST_CAPS = (832, 492, 512, 354)


# Good general cap vectors found by offline search on the canonical routing
# distribution; each is validated against the ACTUAL counts at runtime (DP
# feasibility + placement construction) before use.
_CAPS_CANDIDATES = [(684, 604, 460, 350)]


def _dp_assign(caps, counts_desc):
    """Assign each expert (counts desc) a pair of slot types (i<=j) such
    that caps[i]+caps[j] >= count and each type is used at most 8 times.
    Returns the choice list or None."""
    pairs = [(i, j) for i in range(len(caps)) for j in range(i, len(caps))]
    capsum = {p: caps[p[0]] + caps[p[1]] for p in pairs}
    opts = []
    for c in counts_desc:
        o = [p for p in pairs if capsum[p] >= c]
        if not o:
            return None
        opts.append(o)
    n = len(counts_desc)
    seen = set()
    choice = [None] * n

    def dfs(k, rem):
        if k == n:
            return True
        key = (k, rem)
        if key in seen:
            return False
        for (i, j) in opts[k]:
            r2 = list(rem)
            r2[i] -= 1
            r2[j] -= 1
            if r2[i] >= 0 and r2[j] >= 0:
                choice[k] = (i, j)
                if dfs(k + 1, tuple(r2)):
                    return True
        seen.add(key)
        return False

    if not dfs(0, (NCORES,) * len(caps)):
        return None
    return choice


def _place_from_choice(caps, experts_desc, counts, choice):
    """Build placement[core][slot] = (expert, tok_start, len) from a
    type-pair assignment; both pieces of one expert land on distinct cores.
    Returns placement or None."""
    S = len(caps)
    pieces_per_type = [[] for _ in range(S)]
    for k, e in enumerate(experts_desc):
        i, j = choice[k]
        c = int(counts[e])
        pi = min(caps[i], c)
        pj = c - pi
        pieces_per_type[i].append((e, 0, pi))
        pieces_per_type[j].append((e, pi, pj))
    for rot in range(NCORES):
        placement = [[None] * S for _ in range(NCORES)]
        ok = True
        for t in range(S):
            free = list(range(NCORES))
            free = free[rot:] + free[:rot]
            for (e, st, ln) in sorted(pieces_per_type[t],
                                      key=lambda p: -p[2]):
                cand = [ci for ci in free
                        if e not in {p[0] for p in placement[ci] if p}]
                if not cand:
                    ok = False
                    break
                ci = cand[0]
                placement[ci][t] = (e, st, ln)
                free.remove(ci)
            if not ok:
                break
        if ok:
            return placement
    return None


def _plan_slots(counts: np.ndarray):
    """Choose per-core slot capacities and expert-piece placement.

    Experts are cut into at most two pieces assigned to a pair of slot
    types.  First the precomputed general cap vectors are tried (exact DP
    feasibility on the actual counts); otherwise a threshold-cut search
    (heavy/light primaries + ranked remainders) provides the fallback.

    Returns (caps, placement) where placement[core] is a list of
    (expert, tok_start, length) per slot (length may be 0).
    """
    E_ = len(counts)
    order = np.argsort(-counts, kind="stable")
    heavy = [int(e) for e in order[:NCORES]]
    light = [int(e) for e in order[NCORES:]]
    c0 = int(counts[heavy[0]])
    c8 = int(counts[light[0]])

    def plan_cost(caps):
        ct = sum(caps)
        nch = sum(math.ceil(cp / CH) for cp in caps if cp)
        return 75 * ct + 900 * nch

    def build(A, C):
        pieces_b = []  # (expert, start, len) remainders
        for e in heavy:
            if counts[e] > A:
                pieces_b.append((e, A, int(counts[e]) - A))
        for e in light:
            if counts[e] > C:
                pieces_b.append((e, C, int(counts[e]) - C))
        if len(pieces_b) > 2 * NCORES:
            return None
        pieces_b.sort(key=lambda p: -p[2])
        bs = pieces_b[:NCORES]
        ds = pieces_b[NCORES:]
        a = min(c0, A)
        b = bs[0][2] if bs else 0
        c = min(c8, C)
        d = ds[0][2] if ds else 0
        caps = (a, b, c, d)
        placement = [[None] * 4 for _ in range(NCORES)]
        for i in range(NCORES):
            placement[i][0] = (heavy[i], 0, min(int(counts[heavy[i]]), A))
            placement[i][2] = (light[i], 0, min(int(counts[light[i]]), C))
        for sl, plist in ((1, bs), (3, ds)):
            free = set(range(NCORES))
            for e, st, ln in plist:
                cand = [i for i in free
                        if e != placement[i][0][0] and e != placement[i][2][0]
                        and (placement[i][1] is None or
                             placement[i][1][0] != e)]
                if not cand:
                    return None
                i = cand[0]
                free.discard(i)
                placement[i][sl] = (e, st, ln)
        return caps, placement

    best = None
    lo_a = (c0 + 1) // 2
    lo_c = (c8 + 1) // 2
    cands = [(c0, c8)]
    for A in range(lo_a, c0 + 1, 2):
        for C in range(lo_c, c8 + 1, 2):
            cands.append((A, C))
    for A, C in cands:
        got = build(A, C)
        if got is None:
            continue
        caps, placement = got
        cost = plan_cost(caps)
        if best is None or cost < best[0]:
            best = (cost, caps, placement)
    experts_desc = [int(e) for e in order]
    counts_desc = [int(counts[e]) for e in experts_desc]
    for caps_c in _CAPS_CANDIDATES:
        if plan_cost(caps_c) >= best[0]:
            continue
        choice = _dp_assign(caps_c, counts_desc)
        if choice is None:
            continue
        pl = _place_from_choice(caps_c, experts_desc, counts, choice)
        if pl is None:
            continue
        best = (plan_cost(caps_c), caps_c, pl)
    _, caps, placement = best
    # drop zero-cap slots; fill empty kept slots with a zero-length piece of
    # some expert not already used by that core (perm needs distinct experts)
    keep = [si for si in range(4) if caps[si] > 0]
    caps_k = tuple(caps[si] for si in keep)
    placement_k = []
    for pl in placement:
        row = []
        used = {p[0] for p in pl if p is not None}
        for si in keep:
            p = pl[si]
            if p is None:
                e_fill = next(e for e in range(E_) if e not in used)
                used.add(e_fill)
                p = (e_fill, 0, 0)
            row.append(p)
        placement_k.append(row)
    return caps_k, placement_k


def _chunk_sizes(cap: int) -> list[int]:
    """Split cap into <=256-sized chunks; remainder last."""
    n_full, rem = divmod(cap, CH)
    sizes = [CH] * n_full
    if rem:
        sizes = sizes + [rem]
    return sizes


def _chunk_table(caps):
    """Global chunk list: (slot, t0_in_slot, tl)."""
    out = []
    for s, cap in enumerate(caps):
        t0 = 0
        for tl in _chunk_sizes(cap):
            out.append((s, t0, tl))
            t0 += tl
    return out


def _build_moe(caps: tuple[int, ...]):
    """One-launch fp8 DoubleRow MoE FFN + on-device combine weights.

    Inputs per core (S = len(caps) expert slots, NCH = chunk count):
      w13h/w13l [S, H, 2F] fp8   hi/lo of hstack(w1[e].T, w3[e].T) * SW
      w2h/w2l   [S, F, H]  fp8   hi/lo of w2[e].T * SW
      xph/xpl   [128, NCH, KC, 256] fp8  gathered tokens * SX, packed per
                                   chunk in the SBUF image layout
      gth/gtl   [H, E] fp8       gate_w.T * SG, columns permuted so column s
                                 is slot s's expert
      biasp     [128, E] f32     e_score_correction_bias, same permutation
      identb    [128, 128] bf16  identity (PE transpose)
    Output:
      ygp [128, NCH, FC*2, 256] bf16  combine-weighted expert outputs,
                                 H-on-partitions: ygp[p, c, hk, t] =
                                 y[token t of chunk c, hk*128 + p]
    """
    S = len(caps)
    chunks = _chunk_table(caps)
    NCH = len(chunks)
    ntiles_total = sum(math.ceil(tl / 128) for _, _, tl in chunks)
    HK = H // 128

    nc = bacc.Bacc("TRN2", target_bir_lowering=False, debug=False,
                   num_devices=NCORES)
    w13h = nc.dram_tensor("w13h", [S, 128, KC * 2 * F], FP8,
                          kind="ExternalInput").ap()
    w13l = nc.dram_tensor("w13l", [S, 128, KC * 2 * F], FP8,
                          kind="ExternalInput").ap()
    w2h = nc.dram_tensor("w2h", [S, 128, FC * H], FP8,
                         kind="ExternalInput").ap()
    w2l = nc.dram_tensor("w2l", [S, 128, FC * H], FP8,
                         kind="ExternalInput").ap()
    xph = nc.dram_tensor("xph", [128, NCH, KC * CH], FP8,
                         kind="ExternalInput").ap()
    xpl = nc.dram_tensor("xpl", [128, NCH, KC * CH], FP8,
                         kind="ExternalInput").ap()
    gth = nc.dram_tensor("gth", [128, KC, E], FP8,
                         kind="ExternalInput").ap()
    gtl = nc.dram_tensor("gtl", [128, KC, E], FP8,
                         kind="ExternalInput").ap()
    biasp = nc.dram_tensor("biasp", [128, E], F32, kind="ExternalInput").ap()
    identb = nc.dram_tensor("identb", [128, 128], BF16,
                            kind="ExternalInput").ap()
    ygp = nc.dram_tensor("ygp", [128, NCH, HK * CH], BF16,
                         kind="ExternalOutput").ap()

    SIG = mybir.ActivationFunctionType.Sigmoid
    SILU = mybir.ActivationFunctionType.Silu
    COPY = mybir.ActivationFunctionType.Copy
    MUL = mybir.AluOpType.mult
    ADD = mybir.AluOpType.add

    with tile.TileContext(nc) as tc:
        with (
            tc.tile_pool(name="const_p", bufs=1) as const_p,
            tc.tile_pool(name="w13_p", bufs=2) as w13_p,
            tc.tile_pool(name="w2_p", bufs=2) as w2_p,
            tc.tile_pool(name="xg_p", bufs=3) as xg_p,
            tc.tile_pool(name="ht_p", bufs=2) as ht_p,
            tc.tile_pool(name="work_p", bufs=2) as work_p,
            tc.tile_pool(name="y_p", bufs=3) as y_p,
            tc.tile_pool(name="ps1", bufs=4, space="PSUM") as ps1,
            tc.tile_pool(name="psy", bufs=2, space="PSUM") as psy,
            tc.tile_pool(name="psmisc", bufs=2, space="PSUM") as psmisc,
        ):
            gt_h = const_p.tile([128, KC, E], FP8)
            gt_l = const_p.tile([128, KC, E], FP8)
            bias_sb = const_p.tile([128, E], F32)
            ident_sb = const_p.tile([128, 128], BF16)
            ones_sb = const_p.tile([1, 128], BF16)
            w_sb = const_p.tile([128, ntiles_total], BF16)
            nc.gpsimd.memset(ones_sb[:], 1.0)

            # ---------------- device-side emission helpers ----------------
            def routing(xh, xl, tl, nt, s, cg, jg):
                """Combine weight (bf16, *INV_Y) of slot s's expert for one
                chunk; token-on-partition orientation."""
                ps_r = psmisc.tile([128, 2, E], F32, tag="psmisc",
                                   name=f"ps_r_{cg}")
                nc.vector.memset(ps_r[:, :nt, :], 0.0)
                for j in range(nt):
                    tt0 = j * 128
                    ttl = min(128, tl - tt0)
                    i = 0
                    for (X, G) in ((xh, gt_h), (xl, gt_h), (xh, gt_l)):
                        for kp in range(KC // 2):
                            nc.tensor.matmul(
                                ps_r[:ttl, j, :],
                                lhsT=X[:, 2 * kp:2 * kp + 2, tt0:tt0 + ttl],
                                rhs=G[:, 2 * kp:2 * kp + 2, :],
                                start=(i == 0), stop=(i == 3 * KC // 2 - 1),
                                perf_mode=DR)
                            i += 1
                # sigmoid via tanh: scores = 0.5 + 0.5*tanh(l/2); Tanh shares
                # the activation table with Silu (no LoadActFuncSet thrash)
                th = work_p.tile([128, 2, E], F32, tag="th",
                                 name=f"th_{cg}")
                nc.scalar.activation(th[:, :nt, :], ps_r[:, :nt, :],
                                     mybir.ActivationFunctionType.Tanh,
                                     scale=INVR * 0.5)
                sc = work_p.tile([128, 2, E], F32, tag="sc",
                                 name=f"sc_{cg}")
                nc.vector.tensor_scalar(
                    sc[:, :nt, :], th[:, :nt, :], 0.5, 0.5,
                    op0=MUL, op1=ADD)
                biased = work_p.tile([128, 2, E], F32, tag="biased",
                                     name=f"biased_{cg}")
                nc.vector.tensor_tensor(
                    biased[:, :nt, :], sc[:, :nt, :],
                    bias_sb[:, None, :].to_broadcast([128, nt, E]),
                    ADD)
                m8 = work_p.tile([128, 2, 8], F32, tag="m8",
                                 name=f"m8_{cg}")
                sel = work_p.tile([128, 2, E], F32, tag="sel",
                                  name=f"sel_{cg}")
                for j in range(nt):
                    nc.vector.max(m8[:, j, :], biased[:, j, :])
                for j in range(nt):
                    nc.vector.tensor_scalar(
                        sel[:, j, :], biased[:, j, :],
                        m8[:, j, TOPK - 1:TOPK], None,
                        op0=mybir.AluOpType.is_ge)
                picked = work_p.tile([128, 2, E], F32, tag="picked",
                                     name=f"picked_{cg}")
                nc.vector.tensor_mul(
                    picked[:, :nt, :], sel[:, :nt, :], sc[:, :nt, :])
                denom = work_p.tile([128, 2], F32, tag="denom",
                                    name=f"denom_{cg}")
                nc.vector.reduce_sum(
                    denom[:, :nt], picked[:, :nt, :],
                    axis=mybir.AxisListType.X)
                recip = work_p.tile([128, 2], F32, tag="recip",
                                    name=f"recip_{cg}")
                nc.vector.reciprocal(recip[:, :nt], denom[:, :nt])
                nc.vector.scalar_tensor_tensor(
                    w_sb[:, jg:jg + nt], sc[:, :nt, s], INV_Y,
                    recip[:, :nt], MUL, MUL)

            def evac_stage1(ps_g, ps_u, hth, htl, fi, tl, cg):
                """h*SH in fp8 hi+lo: hi = Q(t), lo = Q(t - hi)."""
                sg = work_p.tile([128, CH], F32, tag="sg",
                                 name=f"sg_{cg}_{fi}")
                nc.scalar.activation(sg[:, :tl], ps_g[:, :tl], SILU,
                                     scale=INV1)
                tt = work_p.tile([128, CH], F32, tag="tt",
                                 name=f"tt_{cg}_{fi}")
                nc.vector.scalar_tensor_tensor(
                    tt[:, :tl], ps_u[:, :tl], C_H, sg[:, :tl], MUL, MUL)
                # Pool does the fp8 cast + lo residual (SBUF-only operands)
                nc.gpsimd.tensor_copy(hth[:, fi, :tl], tt[:, :tl])
                nc.gpsimd.tensor_tensor(
                    htl[:, fi, :tl], tt[:, :tl], hth[:, fi, :tl],
                    mybir.AluOpType.subtract)

            def stage1_fi(ps_g, ps_u, xh, xl, w13h_sb, w13l_sb, fi, tl):
                """ps_g/ps_u accumulate 3 residual terms x 4 k-pairs."""
                i = 0
                for (X, W) in ((xh, w13h_sb), (xl, w13h_sb), (xh, w13l_sb)):
                    for kp in range(KC // 2):
                        nc.tensor.matmul(
                            ps_g[:, :tl],
                            lhsT=W[:, 2 * kp:2 * kp + 2,
                                   fi * 128:(fi + 1) * 128],
                            rhs=X[:, 2 * kp:2 * kp + 2, :tl],
                            start=(i == 0), stop=(i == 3 * KC // 2 - 1),
                            perf_mode=DR)
                        i += 1
                i = 0
                for (X, W) in ((xh, w13h_sb), (xl, w13h_sb), (xh, w13l_sb)):
                    for kp in range(KC // 2):
                        nc.tensor.matmul(
                            ps_u[:, :tl],
                            lhsT=W[:, 2 * kp:2 * kp + 2,
                                   F + fi * 128:F + (fi + 1) * 128],
                            rhs=X[:, 2 * kp:2 * kp + 2, :tl],
                            start=(i == 0), stop=(i == 3 * KC // 2 - 1),
                            perf_mode=DR)
                        i += 1

            def make_wb(p):
                """Combine-weight broadcast along partitions (bf16):
                transpose w column -> rank-1 ones matmul."""
                tl, nt, cg, jg = p["tl"], p["nt"], p["cg"], p["jg"]
                wb_ps = psmisc.tile([128, CH], F32, tag="psmisc",
                                    name=f"wb_ps_{cg}")
                for j in range(nt):
                    wrow_ps = psmisc.tile([1, 128], BF16, tag="psmisc",
                                          name=f"wrow_ps_{cg}_{j}")
                    nc.tensor.transpose(
                        wrow_ps[0:1, :], w_sb[:, jg + j:jg + j + 1],
                        ident_sb[:])
                    wrow_sb = work_p.tile([1, 128], BF16, tag="wrow_sb",
                                          name=f"wrow_sb_{cg}_{j}")
                    nc.vector.tensor_copy(wrow_sb[0:1, :], wrow_ps[0:1, :])
                    nc.tensor.matmul(
                        wb_ps[:, j * 128:j * 128 + 128],
                        lhsT=ones_sb[0:1, :], rhs=wrow_sb[0:1, :],
                        start=True, stop=True)
                wb_sb = work_p.tile([128, CH], BF16, tag="wb_sb",
                                    name=f"wb_sb_{cg}")
                nc.scalar.activation(wb_sb[:, :tl], wb_ps[:, :tl], COPY)
                p["wb_sb"] = wb_sb

            def emit_stage2(p):
                """Stage 2 for one chunk (one behind): H-on-partitions,
                y[p + 128*hk, t] = wb[t] * sum_f h[f,t] w2T[f, p+128*hk]."""
                tl, nt, cg, jg = p["tl"], p["nt"], p["cg"], p["jg"]
                hth, htl = p["hth"], p["htl"]
                w2h_sb, w2l_sb = p["w2h"], p["w2l"]
                if "wb_sb" not in p:
                    make_wb(p)
                wb_sb = p["wb_sb"]
                y_sb = y_p.tile([128, HK, CH], BF16, tag="y",
                                name=f"y_sb_{cg}")
                if tl < CH:
                    # keep the DMA'd tail defined (full-chunk writes)
                    nc.gpsimd.memset(y_sb[:, :, tl:], 0.0)
                for hk in range(HK):
                    ps_y = psy.tile([128, CH], F32, tag="psy",
                                    name=f"ps_y_{cg}_{hk}")
                    i = 0
                    for (A, B) in ((w2h_sb, hth), (w2h_sb, htl),
                                   (w2l_sb, hth)):
                        for kfp in range(FC // 2):
                            nc.tensor.matmul(
                                ps_y[:, :tl],
                                lhsT=A[:, 2 * kfp:2 * kfp + 2,
                                       hk * 128:(hk + 1) * 128],
                                rhs=B[:, 2 * kfp:2 * kfp + 2, :tl],
                                start=(i == 0), stop=(i == 3 * FC // 2 - 1),
                                perf_mode=DR)
                            i += 1
                    if ((p.get("final", False) or p.get("penult", False))
                            and hk % 2 == 1):
                        yr = work_p.tile([128, CH], BF16, tag="yr",
                                         name=f"yr_{cg}_{hk}")
                        nc.scalar.activation(yr[:, :tl], ps_y[:, :tl], COPY)
                        nc.gpsimd.tensor_tensor(
                            y_sb[:, hk, :tl], yr[:, :tl], wb_sb[:, :tl],
                            MUL)
                    else:
                        nc.vector.tensor_tensor(
                            y_sb[:, hk, :tl], ps_y[:, :tl], wb_sb[:, :tl],
                            MUL)
                if p.get("final", False) or p.get("penult", False):
                    # drain fast: per-hk DMAs fire as soon as rows are ready,
                    # alternating issue queues
                    for hk in range(HK):
                        eng = nc.sync if hk % 2 == 0 else nc.scalar
                        eng.dma_start(ygp[:, cg, hk * CH:(hk + 1) * CH],
                                      y_sb[:, hk, :])
                    return None
                return (cg, y_sb)

            # ---------------- weight DMA piece scheduling ----------------
            def w13_tiles(s):
                th = w13_p.tile([128, KC, 2 * F], FP8, tag="w13h",
                                name=f"w13h_sb_{s}")
                tl_ = w13_p.tile([128, KC, 2 * F], FP8, tag="w13l",
                                 name=f"w13l_sb_{s}")
                return th, tl_

            def w2_tiles(s):
                th = w2_p.tile([128, FC, H], FP8, tag="w2h",
                               name=f"w2h_sb_{s}")
                tl_ = w2_p.tile([128, FC, H], FP8, tag="w2l",
                                name=f"w2l_sb_{s}")
                return th, tl_

            def weight_thunks(s, th13, tl13, th2, tl2, pieces=False):
                """DMA thunk list for slot s's weights, in issue order.
                pieces=True splits w13 per k-pair (ramp streaming)."""
                thunks = []
                PW = 2 * 2 * F
                if pieces:
                    for kp in range(KC // 2):
                        thunks.append(
                            lambda kp=kp: nc.gpsimd.dma_start(
                                th13[:, 2 * kp:2 * kp + 2, :].rearrange(
                                    "p k f -> p (k f)"),
                                w13h[s, :, kp * PW:(kp + 1) * PW]))
                    for kp in range(KC // 2):
                        thunks.append(
                            lambda kp=kp: nc.sync.dma_start(
                                tl13[:, 2 * kp:2 * kp + 2, :].rearrange(
                                    "p k f -> p (k f)"),
                                w13l[s, :, kp * PW:(kp + 1) * PW]))
                else:
                    thunks.append(lambda: nc.gpsimd.dma_start(
                        th13[:].rearrange("p k f -> p (k f)"), w13h[s]))
                    thunks.append(lambda: nc.scalar.dma_start(
                        th2[:].rearrange("p k h -> p (k h)"), w2h[s]))
                    thunks.append(lambda: nc.sync.dma_start(
                        tl13[:].rearrange("p k f -> p (k f)"), w13l[s]))
                    thunks.append(lambda: nc.scalar.dma_start(
                        tl2[:].rearrange("p k h -> p (k h)"), w2l[s]))
                    return thunks
                thunks.append(lambda: nc.scalar.dma_start(
                    th2[:].rearrange("p k h -> p (k h)"), w2h[s]))
                thunks.append(lambda: nc.scalar.dma_start(
                    tl2[:].rearrange("p k h -> p (k h)"), w2l[s]))
                return thunks

            # ---------------- main emission ----------------
            slot_w13 = {}
            slot_w2 = {}
            pending = None
            pending_y = None
            prefetch: list = []
            next_x = None
            jglob = 0
            for cg, (s, t0s, tl) in enumerate(chunks):
                nt = math.ceil(tl / 128)
                last_chunks_of_slot = sum(1 for c2 in chunks[cg:]
                                          if c2[0] == s)
                ramp = (cg == 0)

                if s == 0 and cg == 0:
                    slot_w13[0] = w13_tiles(0)
                    slot_w2[0] = w2_tiles(0)
                # s > 0: tiles were created when prefetch was scheduled
                w13h_sb, w13l_sb = slot_w13[s]
                w2h_sb, w2l_sb = slot_w2[s]

                if pending_y is not None:
                    ycg, y_prev = pending_y
                    nc.gpsimd.dma_start(
                        ygp[:, ycg, :],
                        y_prev[:].rearrange("p h c -> p (h c)"))
                    pending_y = None
                if next_x is not None:
                    xh, xl = next_x
                else:
                    xh = xg_p.tile([128, KC, CH], FP8, tag="xgh",
                                   name=f"xh_{cg}")
                    xl = xg_p.tile([128, KC, CH], FP8, tag="xgl",
                                   name=f"xl_{cg}")

                if ramp:
                    # ramp-critical stream: interleave x / w13h k-pair pieces
                    # so the hi-term matmuls start as early as possible
                    PW = 2 * 2 * F
                    PX = 2 * CH
                    for kp in range(KC // 2):
                        nc.sync.dma_start(
                            xh[:, 2 * kp:2 * kp + 2, :].rearrange(
                                "p k c -> p (k c)"),
                            xph[:, 0, kp * PX:(kp + 1) * PX])
                        nc.gpsimd.dma_start(
                            w13h_sb[:, 2 * kp:2 * kp + 2, :].rearrange(
                                "p k f -> p (k f)"),
                            w13h[0, :, kp * PW:(kp + 1) * PW])
                    for kp in range(KC // 2):
                        nc.sync.dma_start(
                            xl[:, 2 * kp:2 * kp + 2, :].rearrange(
                                "p k c -> p (k c)"),
                            xpl[:, 0, kp * PX:(kp + 1) * PX])
                        nc.gpsimd.dma_start(
                            w13l_sb[:, 2 * kp:2 * kp + 2, :].rearrange(
                                "p k f -> p (k f)"),
                            w13l[0, :, kp * PW:(kp + 1) * PW])
                    nc.sync.dma_start(gt_h[:], gth)
                    nc.sync.dma_start(gt_l[:], gtl)
                    nc.sync.dma_start(bias_sb[:], biasp)
                    nc.sync.dma_start(ident_sb[:], identb)
                    # stage 1 as hi-sweep then lo-sweep over ALL fi
                    # (8 psum banks across both pools), k-outer inside so
                    # the matmuls chase the two parallel DMA streams
                    psg = {}
                    psu = {}
                    for fi in range(4):
                        pool = ps1 if fi < 2 else psy
                        tag = "ps1" if fi < 2 else "psy"
                        psg[fi] = pool.tile([128, CH], F32, tag=tag,
                                            name=f"ps_g_r{fi}")
                        psu[fi] = pool.tile([128, CH], F32, tag=tag,
                                            name=f"ps_u_r{fi}")
                    for kp in range(KC // 2):
                        for fi in range(4):
                            for half, ps in ((0, psg[fi]), (1, psu[fi])):
                                nc.tensor.matmul(
                                    ps[:, :tl],
                                    lhsT=w13h_sb[:, 2 * kp:2 * kp + 2,
                                                 half * F + fi * 128:
                                                 half * F + (fi + 1) * 128],
                                    rhs=xh[:, 2 * kp:2 * kp + 2, :tl],
                                    start=(kp == 0), stop=False,
                                    perf_mode=DR)
                    nc.scalar.dma_start(
                        w2h_sb[:].rearrange("p k h -> p (k h)"), w2h[0])
                    nc.scalar.dma_start(
                        w2l_sb[:].rearrange("p k h -> p (k h)"), w2l[0])
                    for ti, (X, W) in enumerate(
                            ((xl, w13h_sb), (xh, w13l_sb))):
                        for kp in range(KC // 2):
                            for fi in range(4):
                                for half, ps in ((0, psg[fi]),
                                                 (1, psu[fi])):
                                    nc.tensor.matmul(
                                        ps[:, :tl],
                                        lhsT=W[:, 2 * kp:2 * kp + 2,
                                               half * F + fi * 128:
                                               half * F + (fi + 1) * 128],
                                        rhs=X[:, 2 * kp:2 * kp + 2, :tl],
                                        start=False,
                                        stop=(ti == 1 and
                                              kp == KC // 2 - 1),
                                        perf_mode=DR)
                    hth = ht_p.tile([128, FC, CH], FP8, tag="hth",
                                    name=f"hth_{cg}")
                    htl = ht_p.tile([128, FC, CH], FP8, tag="htl",
                                    name=f"htl_{cg}")
                    for fi in range(4):
                        evac_stage1(psg[fi], psu[fi], hth, htl, fi, tl, cg)
                    routing(xh, xl, tl, nt, s, cg, jglob)
                else:
                    routing(xh, xl, tl, nt, s, cg, jglob)
                    hth = ht_p.tile([128, FC, CH], FP8, tag="hth",
                                    name=f"hth_{cg}")
                    htl = ht_p.tile([128, FC, CH], FP8, tag="htl",
                                    name=f"htl_{cg}")
                    for fi in range(2):
                        ps_g = ps1.tile([128, 512], F32, tag="ps1",
                                        name=f"ps_g_{cg}_{fi}")
                        ps_u = ps1.tile([128, 512], F32, tag="ps1",
                                        name=f"ps_u_{cg}_{fi}")
                        stage1_fi(ps_g, ps_u, xh, xl, w13h_sb, w13l_sb,
                                  fi, tl)
                        evac_stage1(ps_g, ps_u, hth, htl, fi, tl, cg)
                    if pending is not None:
                        pending_y = emit_stage2(pending)
                    for fi in range(2, 4):
                        ps_g = ps1.tile([128, 512], F32, tag="ps1",
                                        name=f"ps_g_{cg}_{fi}")
                        ps_u = ps1.tile([128, 512], F32, tag="ps1",
                                        name=f"ps_u_{cg}_{fi}")
                        stage1_fi(ps_g, ps_u, xh, xl, w13h_sb, w13l_sb,
                                  fi, tl)
                        evac_stage1(ps_g, ps_u, hth, htl, fi, tl, cg)

                # prefetch: next chunk's tokens (SP queue)
                if cg + 1 < NCH:
                    xh_n = xg_p.tile([128, KC, CH], FP8, tag="xgh",
                                     name=f"xh_{cg + 1}")
                    xl_n = xg_p.tile([128, KC, CH], FP8, tag="xgl",
                                     name=f"xl_{cg + 1}")
                    nc.sync.dma_start(
                        xh_n[:].rearrange("p k c -> p (k c)"),
                        xph[:, cg + 1, :])
                    nc.sync.dma_start(
                        xl_n[:].rearrange("p k c -> p (k c)"),
                        xpl[:, cg + 1, :])
                    next_x = (xh_n, xl_n)
                else:
                    next_x = None

                # prefetch: next slot's weights, spread over this slot's
                # remaining chunks (ACT queue)
                nxt = s + 1
                if nxt < S and nxt not in slot_w13:
                    slot_w13[nxt] = w13_tiles(nxt)
                    slot_w2[nxt] = w2_tiles(nxt)
                    prefetch = weight_thunks(nxt, *slot_w13[nxt],
                                             *slot_w2[nxt])
                if prefetch:
                    npop = math.ceil(len(prefetch) / last_chunks_of_slot) \
                        if last_chunks_of_slot else len(prefetch)
                    for t in prefetch[:npop]:
                        t()
                    prefetch = prefetch[npop:]

                pending = {"tl": tl, "nt": nt, "cg": cg, "jg": jglob,
                           "hth": hth, "htl": htl, "final": cg == NCH - 1,
                           "penult": cg == NCH - 2,
                           "w2h": w2h_sb, "w2l": w2l_sb}
                jglob += nt
                if cg == NCH - 1:
                    # final chunk: wb build as soon as its routing chain is
                    # done (during its own stage 1) to shorten the drain
                    make_wb(pending)

            if pending_y is not None:
                ycg, y_prev = pending_y
                nc.scalar.dma_start(
                    ygp[:, ycg, :],
                    y_prev[:].rearrange("p h c -> p (h c)"))
            emit_stage2(pending)

    nc.compile()
    return nc


def _moe_nc(caps):
    key = ("moe8", caps)
    if key not in _nc_cache:
        _nc_cache[key] = _build_moe(caps)
    return _nc_cache[key]


def _split_fp8(v: np.ndarray, s: float):
    """hi = Q(clip(v*s)), lo = Q(v*s - hi); shared scaled domain."""
    vs = v * s
    hi = np.clip(vs, -240.0, 240.0).astype(NF8)
    lo = (vs - hi.astype(np.float32)).astype(NF8)
    return hi, lo


def kernel(hidden_states, gate_w, bias, w1, w3, w2):
    x = np.ascontiguousarray(np.asarray(hidden_states, dtype=np.float32))
    gate_w = np.asarray(gate_w, dtype=np.float32)
    bias = np.asarray(bias, dtype=np.float32)
    w1 = np.asarray(w1, dtype=np.float32)
    w3 = np.asarray(w3, dtype=np.float32)
    w2 = np.asarray(w2, dtype=np.float32)

    # ---- Host dispatch: fp32 routing decides token->expert placement ----
    logits = x @ gate_w.T                                # [T, E]
    scores = 1.0 / (1.0 + np.exp(-logits))
    biased = scores + bias[None, :]
    topi = np.argpartition(-biased, TOPK - 1, axis=1)[:, :TOPK]  # [T, K]
    sel = np.zeros((T, E), dtype=bool)
    sel[np.arange(T)[:, None], topi] = True
    idx_per_e = [np.nonzero(sel[:, e])[0] for e in range(E)]
    counts = np.array([len(ix) for ix in idx_per_e])
    caps, placement = _plan_slots(counts)
    S = len(caps)
    offs = [sum(caps[:si]) for si in range(S)]
    global LAST_CAPS
    LAST_CAPS = caps
    CT = sum(caps)
    chunks = _chunk_table(caps)
    NCH = len(chunks)

    xT = np.ascontiguousarray(x.T)                       # [H, T]
    gT = np.ascontiguousarray(gate_w.T)                  # [H, E]

    in_maps = []
    for c in range(NCORES):
        slot_experts = [p[0] for p in placement[c]]
        idx_pad = np.zeros(CT, dtype=np.int64)
        for si, (e, st, ln) in enumerate(placement[c]):
            if ln:
                idx_pad[offs[si]:offs[si] + ln] = idx_per_e[e][st:st + ln]
        xg = xT[:, idx_pad]                              # [H, CT] f32
        xg_hi, xg_lo = _split_fp8(xg, SX)
        xph = np.zeros((128, NCH, KC, CH), dtype=NF8)
        xpl = np.zeros((128, NCH, KC, CH), dtype=NF8)
        for cg, (si, t0, tl) in enumerate(chunks):
            colr = slice(offs[si] + t0, offs[si] + t0 + tl)
            xph[:, cg, :, :tl] = \
                xg_hi[:, colr].reshape(KC, 128, tl).transpose(1, 0, 2)
            xpl[:, cg, :, :tl] = \
                xg_lo[:, colr].reshape(KC, 128, tl).transpose(1, 0, 2)

        w13 = np.stack([
            np.concatenate([w1[e].T, w3[e].T], axis=1)
            for e in slot_experts])                      # [S, H, 2F]
        # partition-major SBUF image: [S, 128, KC, 2F]
        w13 = np.ascontiguousarray(
            w13.reshape(S, KC, 128, 2 * F).transpose(0, 2, 1, 3))
        w13h_a, w13l_a = _split_fp8(w13, SW)
        w2t = np.stack([w2[e].T for e in slot_experts])  # [S, F, H]
        w2t = np.ascontiguousarray(
            w2t.reshape(S, FC, 128, H).transpose(0, 2, 1, 3))
        w2h_a, w2l_a = _split_fp8(w2t, SW)

        perm = slot_experts + [e for e in range(E) if e not in slot_experts]
        gtp = np.ascontiguousarray(
            gT[:, perm].reshape(KC, 128, E).transpose(1, 0, 2))
        gth_a, gtl_a = _split_fp8(gtp, SG)
        biasp = np.ascontiguousarray(
            np.broadcast_to(np.asarray(bias)[perm][None, :],
                            (128, E))).astype(np.float32)
        in_maps.append({
            "w13h": w13h_a.reshape(S, 128, KC * 2 * F),
            "w13l": w13l_a.reshape(S, 128, KC * 2 * F),
            "w2h": w2h_a.reshape(S, 128, FC * H),
            "w2l": w2l_a.reshape(S, 128, FC * H),
            "xph": xph.reshape(128, NCH, KC * CH),
            "xpl": xpl.reshape(128, NCH, KC * CH),
            "gth": gth_a, "gtl": gtl_a,
            "biasp": biasp, "identb": np.eye(128, dtype=ml_dtypes.bfloat16),
        })

    # ---- Single SPMD launch: routing weights + expert FFN ----
    ncB = _moe_nc(caps)
    res = run_bass_kernel_spmd(ncB, in_maps, core_ids=list(range(NCORES)))

    # ---- Host combine: scatter-add ----
    out = np.zeros((T, H), dtype=np.float32)
    for c in range(NCORES):
        ygp_c = res.results[c]["ygp"].reshape(128, NCH, H // 128, CH)
        for cg, (si, t0, tl) in enumerate(chunks):
            e, st, ln = placement[c][si]
            cnt = min(max(ln - t0, 0), tl)
            if cnt <= 0:
                continue
            ix = idx_per_e[e][st + t0:st + t0 + cnt]
            blk = ygp_c[:, cg, :, :cnt].astype(np.float32)   # [128, 8, cnt]
            out[ix] += blk.transpose(2, 1, 0).reshape(cnt, H)
    return out
